# revision 5
# baseline (speedup 1.0000x reference)
"""Distributed causal attention head on 8 TRN2 NeuronCores.

Problem: B=4, S=4096, D_in=512, D_out=64 causal attention
  K/V/Q = X @ W; scores = Q@K^T (causal, /sqrt(64)); Z = softmax(scores)@V

Sharding: core c = 2*b + h handles batch b, seq-half h.
q-rows are interleaved at 128-row-block granularity (core h owns global
q-blocks {2j+h}), which makes the causal block schedule IDENTICAL on all
cores (SPMD-safe) and balances FLOPs exactly.  Every core loads the full
(transposed) K/V inputs of its batch and projects them locally.

v3 structure:
 - One flat software pipeline at score-group granularity.  Per group:
   ST pair (row-tiled K=64 matmuls) -> exp on ACT (trimmed to the causal
   trapezoid) -> small diagonal mask multiply on DVE -> AV accumulate
   (LAG groups behind).  All other PE work (next-chunk projections,
   V transposes, previous-chunk normalization) is spread between groups
   through a filler queue so no serial per-chunk phase exists.
 - Causal trapezoid trimming, conservative over both h parities so the
   program is SPMD-identical; per-core mask8 input resolves h.
 - Input DMA on the sync queue in exact first-use order (weights, xq-lo,
   xk/xv of kc0-1, kc2-3, then the second half); K projections for chunk
   0 are unpaired so STs never wait on xv.
 - PE warmup matmuls + early ACT table load during the DMA wait.
 - K/V projections for chunks >= 1 run as column-tiled pairs; V staging
   and transposes live in the upper partition half (id2 identity there).
 - Host permutes K/V columns within each 512-chunk ([j0 j2 j1 j3]) so
   parity-packed kpT copies are two contiguous [64,256] copies.
Matmul inputs bf16, psum/softmax f32; exp with scale=1/8 folded, no
max-subtraction (|scores/8| < ~1.5); AV accumulates Z^T in PSUM with a
ones-column in Vp giving the softmax denominator for free.
"""

import numpy as np
import ml_dtypes

import concourse.bass as bass
import concourse.bacc as bacc
import concourse.mybir as mybir
import concourse.tile as tile

B, S, D, E = 4, 4096, 512, 64
PB = 128                      # partition block
NKB = S // PB                 # 32 k-blocks (global)
NLQ = NKB // 2                # 16 local q-blocks per core
NCH = 4                       # q-chunks of 512 per core
CHW = 512                     # q-chunk width
ND = D // PB                  # 4 d-slices
GRP = 2                       # kblocks per exp group
LAG = 6                       # ST->AV software pipeline depth (groups)
NWARM = 18                    # PE warmup matmuls (N=512)
BF16 = mybir.dt.bfloat16
F32 = mybir.dt.float32
NPBF16 = ml_dtypes.bfloat16

# conservative (h-independent) causal column trim for new-kblock m=0..7
C0TAB = [0, 0, 128, 128, 256, 256, 384, 384]
KVPERM = [0, 2, 1, 3]         # host column-block order within each kc
VSLOT = [0, 2, 1, 3]          # staging slot of original block j


def kparity(kb):
    """kblock -> (partition base, chunk idx, col) in parity-packed kpT."""
    return 64 * (kb % 2), kb // 4, PB * ((kb // 2) % 2)


def build_nc():
    nc = bacc.Bacc(None)

    xq_d = nc.declare_dram_parameter("xq", [D, S // 2], BF16, isOutput=False)
    xk_d = nc.declare_dram_parameter("xk", [D, S], BF16, isOutput=False)
    xv_d = nc.declare_dram_parameter("xv", [D, S], BF16, isOutput=False)
    wq_d = nc.declare_dram_parameter("wq", [D, E], BF16, isOutput=False)
    wk_d = nc.declare_dram_parameter("wk", [D, E], BF16, isOutput=False)
    wv_d = nc.declare_dram_parameter("wv", [D, E], BF16, isOutput=False)
    mk_d = nc.declare_dram_parameter("mask8", [8, PB, PB], BF16, isOutput=False)
    id_d = nc.declare_dram_parameter("ident", [PB, PB], F32, isOutput=False)
    id2_d = nc.declare_dram_parameter("ident2", [PB, E], BF16, isOutput=False)
    out_d = nc.declare_dram_parameter("out", [S // 2, E], F32, isOutput=True)

    with tile.TileContext(nc) as tc:
        with tc.tile_pool(name="persist", bufs=1) as pp, \
             tc.tile_pool(name="st_ps", bufs=2, space="PSUM") as stp, \
             tc.tile_pool(name="pj_ps", bufs=2, space="PSUM") as pjp, \
             tc.tile_pool(name="zt_ps", bufs=2, space="PSUM") as ztp, \
             tc.tile_pool(name="work", bufs=2 * LAG + 2) as wp, \
             tc.tile_pool(name="zsp", bufs=2) as zsp, \
             tc.tile_pool(name="rcp", bufs=4) as rcp, \
             tc.tile_pool(name="osb", bufs=2) as op:
            # ---- persistent SBUF tiles ----
            wq_sb = pp.tile([PB, ND * E], BF16, name="wq_sb", tag="wq_sb")
            wk_sb = pp.tile([PB, ND * E], BF16, name="wk_sb", tag="wk_sb")
            wv_sb = pp.tile([PB, ND * E], BF16, name="wv_sb", tag="wv_sb")
            mk_sb = pp.tile([PB, 8 * PB], BF16, name="mk_sb", tag="mk_sb")
            idf_sb = pp.tile([PB, PB], F32, name="idf_sb", tag="idf_sb")
            id2_sb = pp.tile([PB, E], BF16, name="id2_sb", tag="id2_sb")
            zpad = pp.tile([PB, CHW], BF16, name="zpad", tag="zpad")
            aw_in = pp.tile([PB, 8], F32, name="aw_in", tag="aw_in")
            aw_out = pp.tile([PB, 8], F32, name="aw_out", tag="aw_out")
            # inputs: xq halves; xk/xv in first-use slabs (kc01, kc23, kc4-7)
            xq_sb = [[pp.tile([PB, 2 * CHW], BF16, name=f"xq{d}_{g}", tag=f"xq{d}_{g}")
                      for g in range(2)] for d in range(ND)]
            xk_sb = [[pp.tile([PB, w], BF16, name=f"xk{d}_{i}", tag=f"xk{d}_{i}")
                      for i, w in enumerate((1024, 1024, 2048))] for d in range(ND)]
            xv_sb = [[pp.tile([PB, w], BF16, name=f"xv{d}_{i}", tag=f"xv{d}_{i}")
                      for i, w in enumerate((1024, 1024, 2048))] for d in range(ND)]
            # projected tensors, chunked
            qpT = [pp.tile([PB, CHW], BF16, name=f"qpT{c}", tag=f"qpT{c}") for c in range(NCH)]
            kpT = [pp.tile([PB, 2 * PB], BF16, name=f"kpT{c}", tag=f"kpT{c}")
                   for c in range(2 * NCH)]                # parity-packed
            vpT = [pp.tile([PB, CHW], BF16, name=f"vpT{c}", tag=f"vpT{c}")
                   for c in range(2 * NCH)]                # upper half used
            vp = [pp.tile([PB, E + 1], BF16, name=f"vp{s}", tag=f"vp{s}") for s in range(NKB)]

            def kv_slab(x_sb, d, kc):
                """(tile, column offset) holding k-chunk kc of xk/xv."""
                if kc < 2:
                    return x_sb[d][0], CHW * kc
                if kc < 4:
                    return x_sb[d][1], CHW * (kc - 2)
                return x_sb[d][2], CHW * (kc - 4)

            # ---- t=0: ACT table preload + PE warmup on a zero tile ----
            nc.vector.memset(zpad[:], 0.0)
            nc.vector.memset(aw_in[:], 0.0)
            nc.scalar.activation(aw_out[:], aw_in[:],
                                 mybir.ActivationFunctionType.Exp, scale=0.125)
            for i in range(NWARM):
                wu = pjp.tile([PB, CHW], F32, tag="pj")
                nc.tensor.matmul(wu[:], zpad[:, 0:PB], zpad[:],
                                 start=True, stop=True)
            for s in range(NKB):
                nc.gpsimd.memset(vp[s][:], 1.0)   # ones column prefill

            # ---- input DMAs on sync, in exact first-use order ----
            for w_d, w_sb in ((wq_d, wq_sb), (wk_d, wk_sb), (wv_d, wv_sb)):
                nc.sync.dma_start(
                    out=w_sb[:].rearrange("p (d e) -> p d e", e=E),
                    in_=w_d.rearrange("(d p) e -> p d e", p=PB))

            def dma_half0():
                for d in range(ND):
                    nc.sync.dma_start(
                        out=xq_sb[d][0][:],
                        in_=xq_d[PB * d:PB * (d + 1), 0:2 * CHW])
                for slab, x_sb, x_d in ((0, xk_sb, xk_d), (0, xv_sb, xv_d),
                                        (1, xk_sb, xk_d), (1, xv_sb, xv_d)):
                    for d in range(ND):
                        nc.sync.dma_start(
                            out=x_sb[d][slab][:],
                            in_=x_d[PB * d:PB * (d + 1),
                                    2 * CHW * slab:2 * CHW * (slab + 1)])
                nc.gpsimd.dma_start(out=idf_sb[:], in_=id_d[:])
                nc.gpsimd.dma_start(out=id2_sb[:], in_=id2_d[:])
                nc.gpsimd.dma_start(
                    out=mk_sb[:].rearrange("p (m q) -> p m q", q=PB),
                    in_=mk_d.rearrange("m p q -> p m q"))

            def dma_half1():
                for d in range(ND):
                    nc.sync.dma_start(
                        out=xq_sb[d][1][:],
                        in_=xq_d[PB * d:PB * (d + 1), 2 * CHW:4 * CHW])
                for x_sb, x_d in ((xk_sb, xk_d), (xv_sb, xv_d)):
                    for d in range(ND):
                        nc.sync.dma_start(
                            out=x_sb[d][2][:],
                            in_=x_d[PB * d:PB * (d + 1), 4 * CHW:8 * CHW])

            # ---- building blocks ----
            def proj_q(c):
                g = c // 2
                qof = CHW * (c % 2)
                qp_ps = pjp.tile([E, CHW], F32, tag="pj")
                for d in range(ND):
                    nc.tensor.matmul(qp_ps[:], wq_sb[:, E * d:E * (d + 1)],
                                     xq_sb[d][g][:, qof:qof + CHW],
                                     start=(d == 0), stop=(d == ND - 1))
                nc.vector.tensor_copy(qpT[c][0:E, :], qp_ps[:])
                nc.vector.tensor_copy(qpT[c][E:2 * E, :], qpT[c][0:E, :])

            def copy_kpT(kc, ps, pbase):
                nc.vector.tensor_copy(kpT[kc][0:E, :],
                                      ps[pbase:pbase + E, 0:2 * PB])
                nc.vector.tensor_copy(kpT[kc][E:PB, :],
                                      ps[pbase:pbase + E, 2 * PB:4 * PB])

            def proj_k_solo(kc):
                kp_ps = pjp.tile([E, CHW], F32, tag="pj")
                for d in range(ND):
                    t, kof = kv_slab(xk_sb, d, kc)
                    nc.tensor.matmul(kp_ps[:], wk_sb[:, E * d:E * (d + 1)],
                                     t[:, kof:kof + CHW],
                                     start=(d == 0), stop=(d == ND - 1))
                copy_kpT(kc, kp_ps, 0)

            def proj_v_pair(kc0, kc1):
                """V projections for two k-chunks as a column-tiled pair."""
                vv_ps = pjp.tile([PB, CHW], F32, tag="pj")
                for d in range(ND):
                    t0, of0 = kv_slab(xv_sb, d, kc0)
                    t1, of1 = kv_slab(xv_sb, d, kc1)
                    nc.tensor.matmul(vv_ps[0:E, :], wv_sb[:, E * d:E * (d + 1)],
                                     t0[:, of0:of0 + CHW],
                                     start=(d == 0), stop=(d == ND - 1),
                                     skip_group_check=True)
                    nc.tensor.matmul(vv_ps[E:PB, :], wv_sb[:, E * d:E * (d + 1)],
                                     t1[:, of1:of1 + CHW],
                                     start=(d == 0), stop=(d == ND - 1),
                                     skip_group_check=True)
                nc.vector.tensor_copy(vpT[kc0][E:PB, :], vv_ps[0:E, :])
                nc.vector.tensor_copy(vpT[kc1][E:PB, :], vv_ps[E:PB, :])

            def proj_kv_pair(kc):
                """K (partitions 0:64) + V (64:128) of one k-chunk."""
                kv_ps = pjp.tile([PB, CHW], F32, tag="pj")
                for d in range(ND):
                    tk, kof = kv_slab(xk_sb, d, kc)
                    tv, vof = kv_slab(xv_sb, d, kc)
                    nc.tensor.matmul(kv_ps[0:E, :], wk_sb[:, E * d:E * (d + 1)],
                                     tk[:, kof:kof + CHW],
                                     start=(d == 0), stop=(d == ND - 1),
                                     skip_group_check=True)
                    nc.tensor.matmul(kv_ps[E:PB, :], wv_sb[:, E * d:E * (d + 1)],
                                     tv[:, vof:vof + CHW],
                                     start=(d == 0), stop=(d == ND - 1),
                                     skip_group_check=True)
                copy_kpT(kc, kv_ps, 0)
                nc.vector.tensor_copy(vpT[kc][E:PB, :], kv_ps[E:PB, :])

            def vtrans(s):
                kc, j = s // 4, s % 4
                slot = VSLOT[j]
                vt_ps = pjp.tile([PB, E], BF16, tag="pj")
                nc.tensor.transpose(vt_ps[:],
                                    vpT[kc][E:2 * E, PB * slot:PB * (slot + 1)],
                                    id2_sb[E:2 * E, 0:E])
                nc.vector.tensor_copy(vp[s][:, 0:E], vt_ps[:])

            def st_mm(st_ps, ji, kb, c, c0):
                pb, kch, col = kparity(kb)
                nc.tensor.matmul(st_ps[:, CHW * ji + c0:CHW * (ji + 1)],
                                 kpT[kch][pb:pb + E, col:col + PB],
                                 qpT[c][pb:pb + E, c0:CHW],
                                 start=True, stop=True, tile_position=(pb, 0))

            # ---- filler queue: PE-side work spread between score groups ----
            fillers = []

            def pump(n=1):
                for _ in range(min(n, len(fillers))):
                    fillers.pop(0)()

            def make_norm_fillers(c, zs_sb):
                state = {}

                def step(j, c=c, zs_sb=zs_sb, state=state):
                    if "o" not in state:
                        state["o"] = op.tile([PB, NCH * E], F32, name="o_sb", tag="osb")
                    o_sb = state["o"]
                    zn_ps = pjp.tile([PB, E + 1], F32, tag="pj")
                    nc.tensor.transpose(zn_ps[:], zs_sb[:, PB * j:PB * (j + 1)],
                                        idf_sb[0:E + 1, 0:E + 1])
                    rc_sb = rcp.tile([PB, 1], F32, tag="rc")
                    nc.vector.reciprocal(rc_sb[:], zn_ps[:, E:E + 1])
                    nc.vector.tensor_scalar_mul(o_sb[:, E * j:E * (j + 1)],
                                                zn_ps[:, 0:E], rc_sb[:])

                def outdma(c=c, state=state):
                    q0 = CHW * c
                    eng = nc.sync if c == NCH - 1 else nc.gpsimd
                    eng.dma_start(
                        out=out_d[q0:q0 + CHW, :].rearrange(
                            "(j p) e -> p j e", p=PB),
                        in_=state["o"][:].rearrange("p (j e) -> p j e", e=E))

                return [lambda j=j: step(j) for j in range(4)] + [outdma]

            # ---- prologue ----
            dma_half0()
            proj_q(0)
            proj_k_solo(0)
            fillers.append(lambda: proj_k_solo(1))
            fillers.append(lambda: proj_v_pair(0, 1))

            # ---- main pipeline over chunks/groups ----
            for c in range(NCH):
                nkb = 8 * c + 8
                zt_ps = ztp.tile([E + 1, CHW], F32, tag="zt")
                groups = [list(range(i, i + GRP)) for i in range(0, nkb, GRP)]
                pend = []
                ds = {"n": 0}

                def drain_avs(p_et, p_kbs, zt_ps=zt_ps, c=c, nav=nkb, ds=ds):
                    for kb in p_kbs:
                        if kb >= 8 * c:
                            vtrans(kb)
                    for ji, kb in enumerate(p_kbs):
                        c0 = C0TAB[kb - 8 * c] if kb >= 8 * c else 0
                        nc.tensor.matmul(
                            zt_ps[:, c0:CHW], vp[kb][:],
                            p_et[:, CHW * ji + c0:CHW * (ji + 1)],
                            start=(ds["n"] == 0), stop=(ds["n"] == nav - 1),
                            skip_group_check=True)
                        ds["n"] += 1

                if c == 0:
                    dma_half1()
                if c + 1 < NCH:   # next chunk's projections, spread out
                    fillers.append(lambda c=c: proj_q(c + 1))
                    fillers.append(lambda c=c: proj_kv_pair(2 * c + 2))
                    fillers.append(lambda c=c: proj_kv_pair(2 * c + 3))
                for gi, kbs in enumerate(groups):
                    diag = kbs[0] >= 8 * c
                    c0 = C0TAB[kbs[0] - 8 * c] if diag else 0
                    st_ps = stp.tile([PB, GRP * CHW], F32, tag="st")
                    st_mm(st_ps, 0, kbs[0], c, c0)
                    st_mm(st_ps, 1, kbs[1], c, c0)
                    if len(pend) > LAG - 1:
                        drain_avs(*pend.pop(0))
                    et_sb = wp.tile([PB, GRP * CHW], BF16, tag="et")
                    if c0 == 0:
                        nc.scalar.activation(
                            et_sb[:], st_ps[:],
                            mybir.ActivationFunctionType.Exp, scale=0.125)
                    else:
                        nc.scalar.activation(
                            et_sb[:].rearrange("p (u q) -> p u q", q=CHW)[:, :, c0:CHW],
                            st_ps[:].rearrange("p (u q) -> p u q", q=CHW)[:, :, c0:CHW],
                            mybir.ActivationFunctionType.Exp, scale=0.125)
                    if diag:
                        m0 = kbs[0] - 8 * c
                        nc.vector.tensor_mul(
                            et_sb[:].rearrange("p (u q) -> p u q", q=CHW)[:, :, c0:c0 + PB],
                            et_sb[:].rearrange("p (u q) -> p u q", q=CHW)[:, :, c0:c0 + PB],
                            mk_sb[:, PB * m0:PB * (m0 + 2)].rearrange(
                                "p (u q) -> p u q", q=PB))
                    pend.append((et_sb, kbs))
                    pump(1)
                while pend:
                    drain_avs(*pend.pop(0))
                    pump(1)
                zs_sb = zsp.tile([E + 1, CHW], F32, tag="zs")
                nc.vector.tensor_copy(zs_sb[:], zt_ps[:])
                fillers.extend(make_norm_fillers(c, zs_sb))
            # tail: whatever fillers remain (last chunk's norm + out DMA)
            pump(len(fillers))
    nc.finalize()
    return nc


def make_core_inputs(key_np, value_np, query_np, Wk, Wv, Wq):
    """Host-side sharding: returns in_maps list of 8 dicts."""
    bf = lambda a: np.ascontiguousarray(a).astype(NPBF16)
    perm = np.concatenate([512 * kc + np.concatenate(
        [np.arange(128 * j, 128 * j + 128) for j in KVPERM])
        for kc in range(S // 512)])
    ki = np.arange(PB)[:, None]
    qi = np.arange(PB)[None, :]
    tri = (ki <= qi).astype(np.float32)
    ones = np.ones((PB, PB), np.float32)
    zeros = np.zeros((PB, PB), np.float32)
    in_maps = []
    for c in range(8):
        b, h = c // 2, c % 2
        qrows = np.concatenate(
            [np.arange(PB * (2 * j + h), PB * (2 * j + h) + PB) for j in range(NLQ)])
        mask8 = np.zeros((8, PB, PB), dtype=np.float32)
        for m in range(8):
            if m % 2 == 0:
                mask8[m] = tri if h == 0 else ones
            else:
                mask8[m] = zeros if h == 0 else tri
        id2 = np.zeros((PB, E), np.float32)
        id2[E:2 * E, :] = np.eye(E)
        in_maps.append({
            "xq": bf(query_np[b][qrows].T),
            "xk": bf(key_np[b].T[:, perm]),
            "xv": bf(value_np[b].T[:, perm]),
            "wq": bf(Wq), "wk": bf(Wk), "wv": bf(Wv),
            "mask8": bf(mask8),
            "ident": np.eye(PB, dtype=np.float32),
            "ident2": bf(id2),
        })
    return in_maps


def assemble_output(results):
    """results: list of 8 dicts with 'out' [2048, 64] f32 -> Z [B,S,E]."""
    Z = np.zeros((B, S, E), dtype=np.float32)
    for c in range(8):
        b, h = c // 2, c % 2
        o = results[c]["out"]  # [2048, E] q-major
        for j in range(NLQ):
            g = 2 * j + h
            Z[b, PB * g:PB * (g + 1), :] = o[PB * j:PB * (j + 1), :]
    return Z


def kernel(key_inputs, value_inputs, query_inputs, Wk, Wv, Wq):
    from concourse.bass_utils import run_bass_kernel_spmd
    nc = build_nc()
    in_maps = make_core_inputs(np.asarray(key_inputs), np.asarray(value_inputs),
                               np.asarray(query_inputs), np.asarray(Wk),
                               np.asarray(Wv), np.asarray(Wq))
    res = run_bass_kernel_spmd(nc, in_maps, core_ids=list(range(8)))
    return assemble_output(res.results)


# revision 8
# speedup vs baseline: 1.0776x; 1.0776x over previous
"""Distributed causal attention head on 8 TRN2 NeuronCores.

Problem: B=4, S=4096, D_in=512, D_out=64 causal attention
  K/V/Q = X @ W; scores = Q@K^T (causal, /sqrt(64)); Z = softmax(scores)@V

Sharding: core c = 2*b + h handles batch b, seq-half h.
q-rows are interleaved at 128-row-block granularity (core h owns global
q-blocks {2j+h}), which makes the causal block schedule IDENTICAL on all
cores (SPMD-safe) and balances FLOPs exactly.  Every core loads the full
(transposed) K/V inputs of its batch and projects them locally.

v3 structure:
 - One flat software pipeline at score-group granularity.  Per group:
   ST pair (row-tiled K=64 matmuls) -> exp on ACT (trimmed to the causal
   trapezoid) -> small diagonal mask multiply on DVE -> AV accumulate
   (LAG groups behind).  All other PE work (next-chunk projections,
   V transposes, previous-chunk normalization) is spread between groups
   through a filler queue so no serial per-chunk phase exists.
 - Causal trapezoid trimming, conservative over both h parities so the
   program is SPMD-identical; per-core mask8 input resolves h.
 - Input DMA on the sync queue in exact first-use order (weights, xq-lo,
   xk/xv of kc0-1, kc2-3, then the second half); K projections for chunk
   0 are unpaired so STs never wait on xv.
 - PE warmup matmuls + early ACT table load during the DMA wait.
 - K/V projections for chunks >= 1 run as column-tiled pairs; V staging
   and transposes live in the upper partition half (id2 identity there).
 - Host permutes K/V columns within each 512-chunk ([j0 j2 j1 j3]) so
   parity-packed kpT copies are two contiguous [64,256] copies.
Matmul inputs bf16, psum/softmax f32; exp with scale=1/8 folded, no
max-subtraction (|scores/8| < ~1.5); AV accumulates Z^T in PSUM with a
ones-column in Vp giving the softmax denominator for free.
"""

import numpy as np
import ml_dtypes

import concourse.bass as bass
import concourse.bacc as bacc
import concourse.mybir as mybir
import concourse.tile as tile

B, S, D, E = 4, 4096, 512, 64
PB = 128                      # partition block
NKB = S // PB                 # 32 k-blocks (global)
NLQ = NKB // 2                # 16 local q-blocks per core
NCH = 4                       # q-chunks of 512 per core
CHW = 512                     # q-chunk width
ND = D // PB                  # 4 d-slices
GRP = 2                       # kblocks per exp group
LAG = 6                       # ST->AV software pipeline depth (groups)
NWARM = 18                    # PE warmup matmuls (N=512)
BF16 = mybir.dt.bfloat16
F32 = mybir.dt.float32
NPBF16 = ml_dtypes.bfloat16

# conservative (h-independent) causal column trim for new-kblock m=0..7
C0TAB = [0, 0, 128, 128, 256, 256, 384, 384]
KVPERM = [0, 2, 1, 3]         # host column-block order within each kc
VSLOT = [0, 2, 1, 3]          # staging slot of original block j


def kparity(kb):
    """kblock -> (partition base, chunk idx, col) in parity-packed kpT."""
    return 64 * (kb % 2), kb // 4, PB * ((kb // 2) % 2)


def build_nc():
    nc = bacc.Bacc(None)

    xq_d = nc.declare_dram_parameter("xq", [D, S // 2], BF16, isOutput=False)
    xk_d = nc.declare_dram_parameter("xk", [D, S], BF16, isOutput=False)
    xv_d = nc.declare_dram_parameter("xv", [D, S], BF16, isOutput=False)
    wq_d = nc.declare_dram_parameter("wq", [D, E], BF16, isOutput=False)
    wk_d = nc.declare_dram_parameter("wk", [D, E], BF16, isOutput=False)
    wv_d = nc.declare_dram_parameter("wv", [D, E], BF16, isOutput=False)
    mk_d = nc.declare_dram_parameter("mask8", [8, PB, PB], BF16, isOutput=False)
    id_d = nc.declare_dram_parameter("ident", [PB, PB], F32, isOutput=False)
    id2_d = nc.declare_dram_parameter("ident2", [PB, E], BF16, isOutput=False)
    out_d = nc.declare_dram_parameter("out", [S // 2, E], F32, isOutput=True)

    with tile.TileContext(nc) as tc:
        with tc.tile_pool(name="persist", bufs=1) as pp, \
             tc.tile_pool(name="st_ps", bufs=2, space="PSUM") as stp, \
             tc.tile_pool(name="pj_ps", bufs=2, space="PSUM") as pjp, \
             tc.tile_pool(name="zt_ps", bufs=2, space="PSUM") as ztp, \
             tc.tile_pool(name="work", bufs=2 * LAG + 2) as wp, \
             tc.tile_pool(name="zsp", bufs=2) as zsp, \
             tc.tile_pool(name="rcp", bufs=4) as rcp, \
             tc.tile_pool(name="osb", bufs=2) as op:
            # ---- persistent SBUF tiles ----
            wq_sb = pp.tile([PB, ND * E], BF16, name="wq_sb", tag="wq_sb")
            wk_sb = pp.tile([PB, ND * E], BF16, name="wk_sb", tag="wk_sb")
            wv_sb = pp.tile([PB, ND * E], BF16, name="wv_sb", tag="wv_sb")
            mk_sb = pp.tile([PB, 8 * PB], BF16, name="mk_sb", tag="mk_sb")
            idf_sb = pp.tile([PB, PB], F32, name="idf_sb", tag="idf_sb")
            id2_sb = pp.tile([PB, E], BF16, name="id2_sb", tag="id2_sb")
            zpad = pp.tile([PB, CHW], BF16, name="zpad", tag="zpad")
            aw_in = pp.tile([PB, 8], F32, name="aw_in", tag="aw_in")
            aw_out = pp.tile([PB, 8], F32, name="aw_out", tag="aw_out")
            # inputs: xq halves; xk/xv in first-use slabs (kc01, kc23, kc4-7)
            xq_sb = [[pp.tile([PB, 2 * CHW], BF16, name=f"xq{d}_{g}", tag=f"xq{d}_{g}")
                      for g in range(2)] for d in range(ND)]
            xk_sb = [[pp.tile([PB, 4 * CHW], BF16, name=f"xk{d}_{g}", tag=f"xk{d}_{g}")
                      for g in range(2)] for d in range(ND)]
            xv_sb = [[pp.tile([PB, 4 * CHW], BF16, name=f"xv{d}_{g}", tag=f"xv{d}_{g}")
                      for g in range(2)] for d in range(ND)]
            # projected tensors, chunked
            qpT = [pp.tile([PB, CHW], BF16, name=f"qpT{c}", tag=f"qpT{c}") for c in range(NCH)]
            kpT = [pp.tile([PB, 2 * PB], BF16, name=f"kpT{c}", tag=f"kpT{c}")
                   for c in range(2 * NCH)]                # parity-packed
            vpT = [pp.tile([PB, CHW], BF16, name=f"vpT{c}", tag=f"vpT{c}")
                   for c in range(2 * NCH)]                # upper half used
            vp = [pp.tile([PB, E + 1], BF16, name=f"vp{s}", tag=f"vp{s}") for s in range(NKB)]

            def kv_slab(x_sb, d, kc):
                """(tile, column offset) holding k-chunk kc of xk/xv."""
                return x_sb[d][kc // 4], CHW * (kc % 4)

            # ---- t=0: ACT table preload + PE warmup on a zero tile ----
            nc.vector.memset(zpad[:], 0.0)
            nc.vector.memset(aw_in[:], 0.0)
            nc.scalar.activation(aw_out[:], aw_in[:],
                                 mybir.ActivationFunctionType.Exp, scale=0.125)
            for i in range(NWARM):
                wu = pjp.tile([PB, CHW], F32, tag="pj")
                nc.tensor.matmul(wu[:], zpad[:, 0:PB], zpad[:],
                                 start=True, stop=True)
            for s in range(NKB):
                nc.gpsimd.memset(vp[s][:], 1.0)   # ones column prefill

            # ---- input DMAs on sync, in exact first-use order ----
            for w_d, w_sb in ((wq_d, wq_sb), (wk_d, wk_sb), (wv_d, wv_sb)):
                nc.sync.dma_start(
                    out=w_sb[:].rearrange("p (d e) -> p d e", e=E),
                    in_=w_d.rearrange("(d p) e -> p d e", p=PB))

            def dma_half(g):
                for d in range(ND):
                    nc.sync.dma_start(
                        out=xq_sb[d][g][:],
                        in_=xq_d[PB * d:PB * (d + 1), 2 * CHW * g:2 * CHW * (g + 1)])
                for d in range(ND):
                    nc.sync.dma_start(
                        out=xk_sb[d][g][:],
                        in_=xk_d[PB * d:PB * (d + 1), 4 * CHW * g:4 * CHW * (g + 1)])
                for d in range(ND):
                    nc.scalar.dma_start(
                        out=xv_sb[d][g][:],
                        in_=xv_d[PB * d:PB * (d + 1), 4 * CHW * g:4 * CHW * (g + 1)])
                if g == 0:
                    nc.gpsimd.dma_start(out=idf_sb[:], in_=id_d[:])
                    nc.gpsimd.dma_start(out=id2_sb[:], in_=id2_d[:])
                    nc.gpsimd.dma_start(
                        out=mk_sb[:].rearrange("p (m q) -> p m q", q=PB),
                        in_=mk_d.rearrange("m p q -> p m q"))

            # ---- building blocks ----
            def proj_q(c):
                g = c // 2
                qof = CHW * (c % 2)
                qp_ps = pjp.tile([E, CHW], F32, tag="pj")
                for d in range(ND):
                    nc.tensor.matmul(qp_ps[:], wq_sb[:, E * d:E * (d + 1)],
                                     xq_sb[d][g][:, qof:qof + CHW],
                                     start=(d == 0), stop=(d == ND - 1))
                nc.vector.tensor_copy(qpT[c][0:E, :], qp_ps[:])
                nc.vector.tensor_copy(qpT[c][E:2 * E, :], qpT[c][0:E, :])

            def copy_kpT(kc, ps, pbase):
                nc.vector.tensor_copy(kpT[kc][0:E, :],
                                      ps[pbase:pbase + E, 0:2 * PB])
                nc.vector.tensor_copy(kpT[kc][E:PB, :],
                                      ps[pbase:pbase + E, 2 * PB:4 * PB])

            def proj_k_solo(kc):
                kp_ps = pjp.tile([E, CHW], F32, tag="pj")
                for d in range(ND):
                    t, kof = kv_slab(xk_sb, d, kc)
                    nc.tensor.matmul(kp_ps[:], wk_sb[:, E * d:E * (d + 1)],
                                     t[:, kof:kof + CHW],
                                     start=(d == 0), stop=(d == ND - 1))
                copy_kpT(kc, kp_ps, 0)

            def proj_v_pair(kc0, kc1):
                """V projections for two k-chunks as a column-tiled pair."""
                vv_ps = pjp.tile([PB, CHW], F32, tag="pj")
                for d in range(ND):
                    t0, of0 = kv_slab(xv_sb, d, kc0)
                    t1, of1 = kv_slab(xv_sb, d, kc1)
                    nc.tensor.matmul(vv_ps[0:E, :], wv_sb[:, E * d:E * (d + 1)],
                                     t0[:, of0:of0 + CHW],
                                     start=(d == 0), stop=(d == ND - 1),
                                     skip_group_check=True)
                    nc.tensor.matmul(vv_ps[E:PB, :], wv_sb[:, E * d:E * (d + 1)],
                                     t1[:, of1:of1 + CHW],
                                     start=(d == 0), stop=(d == ND - 1),
                                     skip_group_check=True)
                nc.vector.tensor_copy(vpT[kc0][E:PB, :], vv_ps[0:E, :])
                nc.vector.tensor_copy(vpT[kc1][E:PB, :], vv_ps[E:PB, :])

            def proj_kv_pair(kc):
                """K (partitions 0:64) + V (64:128) of one k-chunk."""
                kv_ps = pjp.tile([PB, CHW], F32, tag="pj")
                for d in range(ND):
                    tk, kof = kv_slab(xk_sb, d, kc)
                    tv, vof = kv_slab(xv_sb, d, kc)
                    nc.tensor.matmul(kv_ps[0:E, :], wk_sb[:, E * d:E * (d + 1)],
                                     tk[:, kof:kof + CHW],
                                     start=(d == 0), stop=(d == ND - 1),
                                     skip_group_check=True)
                    nc.tensor.matmul(kv_ps[E:PB, :], wv_sb[:, E * d:E * (d + 1)],
                                     tv[:, vof:vof + CHW],
                                     start=(d == 0), stop=(d == ND - 1),
                                     skip_group_check=True)
                copy_kpT(kc, kv_ps, 0)
                nc.vector.tensor_copy(vpT[kc][E:PB, :], kv_ps[E:PB, :])

            def vtrans(s):
                kc, j = s // 4, s % 4
                slot = VSLOT[j]
                vt_ps = pjp.tile([PB, E], BF16, tag="pj")
                nc.tensor.transpose(vt_ps[:],
                                    vpT[kc][E:2 * E, PB * slot:PB * (slot + 1)],
                                    id2_sb[E:2 * E, 0:E])
                nc.vector.tensor_copy(vp[s][:, 0:E], vt_ps[:])

            def st_mm(st_ps, ji, kb, c, c0):
                pb, kch, col = kparity(kb)
                nc.tensor.matmul(st_ps[:, CHW * ji + c0:CHW * (ji + 1)],
                                 kpT[kch][pb:pb + E, col:col + PB],
                                 qpT[c][pb:pb + E, c0:CHW],
                                 start=True, stop=True, tile_position=(pb, 0))

            # ---- filler queue: PE-side work spread between score groups ----
            fillers = []

            def pump(n=1):
                for _ in range(min(n, len(fillers))):
                    fillers.pop(0)()

            def make_norm_fillers(c, zs_sb):
                state = {}

                def step(j, c=c, zs_sb=zs_sb, state=state):
                    if "o" not in state:
                        state["o"] = op.tile([PB, NCH * E], F32, name="o_sb", tag="osb")
                    o_sb = state["o"]
                    zn_ps = pjp.tile([PB, E + 1], F32, tag="pj")
                    nc.tensor.transpose(zn_ps[:], zs_sb[:, PB * j:PB * (j + 1)],
                                        idf_sb[0:E + 1, 0:E + 1])
                    rc_sb = rcp.tile([PB, 1], F32, tag="rc")
                    nc.vector.reciprocal(rc_sb[:], zn_ps[:, E:E + 1])
                    nc.vector.tensor_scalar_mul(o_sb[:, E * j:E * (j + 1)],
                                                zn_ps[:, 0:E], rc_sb[:])

                def outdma(c=c, state=state):
                    q0 = CHW * c
                    eng = nc.sync if c == NCH - 1 else nc.gpsimd
                    eng.dma_start(
                        out=out_d[q0:q0 + CHW, :].rearrange(
                            "(j p) e -> p j e", p=PB),
                        in_=state["o"][:].rearrange("p (j e) -> p j e", e=E))

                return [lambda j=j: step(j) for j in range(4)] + [outdma]

            # ---- prologue ----
            dma_half(0)
            proj_q(0)
            proj_k_solo(0)
            fillers.append(lambda: proj_k_solo(1))
            fillers.append(lambda: proj_v_pair(0, 1))

            # ---- main pipeline over chunks/groups ----
            # PE work is batched by stationary family (ST block for two
            # groups, then AV block for two groups) so LDWEIGHTS prefetch
            # stays effective; the last chunk runs its diagonal groups
            # first and tapers the AV lag so the tail is short.
            for c in range(NCH):
                nkb = 8 * c + 8
                zt_ps = ztp.tile([E + 1, CHW], F32, tag="zt")
                korder = list(range(nkb))
                if c == NCH - 1:
                    korder = korder[8 * c:] + korder[:8 * c]
                groups = [korder[i:i + GRP] for i in range(0, nkb, GRP)]
                pend = []
                ds = {"n": 0}

                def drain2(zt_ps=zt_ps, c=c, nav=nkb, ds=ds, pend=pend):
                    """Drain up to two pending groups: batched vtrans, then
                    a batched run of AV matmuls."""
                    take = pend[:2]
                    del pend[:2]
                    for p_et, p_kbs in take:
                        for kb in p_kbs:
                            if kb >= 8 * c:
                                vtrans(kb)
                    for p_et, p_kbs in take:
                        for ji, kb in enumerate(p_kbs):
                            c0 = C0TAB[kb - 8 * c] if kb >= 8 * c else 0
                            nc.tensor.matmul(
                                zt_ps[:, c0:CHW], vp[kb][:],
                                p_et[:, CHW * ji + c0:CHW * (ji + 1)],
                                start=(ds["n"] == 0), stop=(ds["n"] == nav - 1),
                                skip_group_check=True)
                            ds["n"] += 1

                def do_group(kbs, c=c):
                    diag = kbs[0] >= 8 * c
                    c0 = C0TAB[kbs[0] - 8 * c] if diag else 0
                    st_ps = stp.tile([PB, GRP * CHW], F32, tag="st")
                    st_mm(st_ps, 0, kbs[0], c, c0)
                    st_mm(st_ps, 1, kbs[1], c, c0)
                    et_sb = wp.tile([PB, GRP * CHW], BF16, tag="et")
                    if c0 == 0:
                        nc.scalar.activation(
                            et_sb[:], st_ps[:],
                            mybir.ActivationFunctionType.Exp, scale=0.125)
                    else:
                        nc.scalar.activation(
                            et_sb[:].rearrange("p (u q) -> p u q", q=CHW)[:, :, c0:CHW],
                            st_ps[:].rearrange("p (u q) -> p u q", q=CHW)[:, :, c0:CHW],
                            mybir.ActivationFunctionType.Exp, scale=0.125)
                    if diag:
                        m0 = kbs[0] - 8 * c
                        nc.vector.tensor_mul(
                            et_sb[:].rearrange("p (u q) -> p u q", q=CHW)[:, :, c0:c0 + PB],
                            et_sb[:].rearrange("p (u q) -> p u q", q=CHW)[:, :, c0:c0 + PB],
                            mk_sb[:, PB * m0:PB * (m0 + 2)].rearrange(
                                "p (u q) -> p u q", q=PB))
                    pend.append((et_sb, kbs))

                if c == 0:
                    dma_half(1)
                if c + 1 < NCH:   # next chunk's projections, spread out
                    fillers.append(lambda c=c: proj_q(c + 1))
                    fillers.append(lambda c=c: proj_kv_pair(2 * c + 2))
                    fillers.append(lambda c=c: proj_kv_pair(2 * c + 3))
                lag = LAG
                for gi in range(0, len(groups), 2):
                    if c == NCH - 1 and gi >= len(groups) - 4:
                        lag = 2   # taper for a short tail
                    do_group(groups[gi])
                    do_group(groups[gi + 1])
                    if len(pend) > lag - 2:
                        drain2()
                    pump(2)
                while pend:
                    drain2()
                    pump(2)
                zs_sb = zsp.tile([E + 1, CHW], F32, tag="zs")
                nc.vector.tensor_copy(zs_sb[:], zt_ps[:])
                fillers.extend(make_norm_fillers(c, zs_sb))
            # tail: whatever fillers remain (last chunk's norm + out DMA)
            pump(len(fillers))
    nc.finalize()
    return nc


def make_core_inputs(key_np, value_np, query_np, Wk, Wv, Wq):
    """Host-side sharding: returns in_maps list of 8 dicts."""
    bf = lambda a: np.ascontiguousarray(a).astype(NPBF16)
    perm = np.concatenate([512 * kc + np.concatenate(
        [np.arange(128 * j, 128 * j + 128) for j in KVPERM])
        for kc in range(S // 512)])
    ki = np.arange(PB)[:, None]
    qi = np.arange(PB)[None, :]
    tri = (ki <= qi).astype(np.float32)
    ones = np.ones((PB, PB), np.float32)
    zeros = np.zeros((PB, PB), np.float32)
    in_maps = []
    for c in range(8):
        b, h = c // 2, c % 2
        qrows = np.concatenate(
            [np.arange(PB * (2 * j + h), PB * (2 * j + h) + PB) for j in range(NLQ)])
        mask8 = np.zeros((8, PB, PB), dtype=np.float32)
        for m in range(8):
            if m % 2 == 0:
                mask8[m] = tri if h == 0 else ones
            else:
                mask8[m] = zeros if h == 0 else tri
        id2 = np.zeros((PB, E), np.float32)
        id2[E:2 * E, :] = np.eye(E)
        in_maps.append({
            "xq": bf(query_np[b][qrows].T),
            "xk": bf(key_np[b].T[:, perm]),
            "xv": bf(value_np[b].T[:, perm]),
            "wq": bf(Wq), "wk": bf(Wk), "wv": bf(Wv),
            "mask8": bf(mask8),
            "ident": np.eye(PB, dtype=np.float32),
            "ident2": bf(id2),
        })
    return in_maps


def assemble_output(results):
    """results: list of 8 dicts with 'out' [2048, 64] f32 -> Z [B,S,E]."""
    Z = np.zeros((B, S, E), dtype=np.float32)
    for c in range(8):
        b, h = c // 2, c % 2
        o = results[c]["out"]  # [2048, E] q-major
        for j in range(NLQ):
            g = 2 * j + h
            Z[b, PB * g:PB * (g + 1), :] = o[PB * j:PB * (j + 1), :]
    return Z


def kernel(key_inputs, value_inputs, query_inputs, Wk, Wv, Wq):
    from concourse.bass_utils import run_bass_kernel_spmd
    nc = build_nc()
    in_maps = make_core_inputs(np.asarray(key_inputs), np.asarray(value_inputs),
                               np.asarray(query_inputs), np.asarray(Wk),
                               np.asarray(Wv), np.asarray(Wq))
    res = run_bass_kernel_spmd(nc, in_maps, core_ids=list(range(8)))
    return assemble_output(res.results)
